# revision 50
# baseline (speedup 1.0000x reference)
"""Trainium2 Bass kernel for nn_MultiHeadSelfAttention_90537910600353.

Reference (B=2, S=2048, E=1024, H=16, d=64):
    L_h   = tril(params[h]);  scores = (x_h Ls)(x_h Ls)^T,  Ls = L/sqrt(8)
    attn  = softmax(scores);  V = x Wv^T + bv
    out   = (attn @ V_h) @ Wo^T + bo

Sharding: batch*head over 8 cores; core m: b = m//4, heads [4(m%4), 4(m%4)+4).
Each core emits partial[S, E] (bf16); host sums 4 partials/batch + bo.

Per-core algorithm (all engine work balanced against the PE roofline):
  z      = x_h Ls                      (bf16 matmul, PSUM f32)
  zh/zl  = fp8e4m3 pair split of z     (Act copy + DVE subtract)
  M      = |z|^2 + 24 via Square + ones-matmul; fp8 pair -(M)/4 rows
  scores = fp8 DoubleRow matmul (0.5 cyc/row): slot0 = zh.zh + zl'.zh - M,
           slot1 = zh.zl  -> exact-ish 3-term pair product
  E      = exp(scores) -> bf16; split between Act (true exp) and DVE
           (Schraudolph uint16 bit-trick exp, validated on silicon)
  attnV  = weights-stationary matmuls: lhsT = E tile [t,q], rhs = V'[t, 65]
           (aug ones col -> denominator), accumulate over t in PSUM [q, 16, 65]
  outn   = num * recip(den) per partition (q) -> bf16 [q, qb, h%2, d]
  onT    = DMA-xbar blocked transpose -> [hd, qb, q]
  partial= onT^T @ Wo^T (bf16) -> PSUM -> bf16 -> DRAM

Self-contained: numpy + ml_dtypes + concourse at /opt/trn_rl_repo.
"""

import sys

if "/opt/trn_rl_repo" not in sys.path:
    sys.path.insert(0, "/opt/trn_rl_repo")

import numpy as np
import ml_dtypes

import concourse.bass as bass
import concourse.mybir as mybir
import concourse.tile as tile
from concourse.bass_utils import run_bass_kernel_spmd

F32 = mybir.dt.float32
BF16 = mybir.dt.bfloat16
F8 = mybir.dt.float8e4
U16 = mybir.dt.uint16
NPF8 = ml_dtypes.float8_e4m3
NPBF = ml_dtypes.bfloat16

B, S, E, H = 2, 2048, 1024, 16
D = 64
NCORES = 8
HPC = 4
NT = S // 128       # 16 t-blocks
MARGIN = 24.0

SCH_SCALE = float(np.log2(np.e) * 128.0)
SCH_BIAS = float(127 * 128 - 10.0)
Y_DVE = 56          # of 128 exp spans go to DVE (Schraudolph)

ACT = mybir.ActivationFunctionType
ALU = mybir.AluOpType


def _split_multi_waits(nc):
    """This walrus build rejects instructions carrying more than one sync
    wait. Hoist extra waits onto same-engine NOPs inserted just before."""
    for f in nc.m.functions:
        for b in f.blocks:
            il = b.instructions
            i = 0
            while i < len(il):
                inst = il[i]
                si = getattr(inst, "sync_info", None)
                if si is not None and si.on_wait and len(si.on_wait) > 1:
                    waits = list(si.on_wait)
                    for w in waits[:-1]:
                        nop = mybir.InstNoOp(
                            name=nc.get_next_instruction_name(),
                            engine=inst.engine,
                            sync_info=mybir.SyncInfo(on_wait=[w], on_update=[]),
                        )
                        il.insert(i, nop)
                        i += 1
                    inst.sync_info = mybir.SyncInfo(
                        on_wait=[waits[-1]], on_update=si.on_update
                    )
                i += 1


def _dve_flags():
    """Bresenham spread of Y_DVE Schraudolph spans over the 128 (h,t,s)."""
    return [((i + 1) * Y_DVE) // 128 - (i * Y_DVE) // 128 == 1 for i in range(128)]


def build_program():
    nc = bass.Bass("TRN2", target_bir_lowering=False, debug=False,
                   num_devices=NCORES)

    xt = nc.dram_tensor("xt", [8, 128, S], BF16, kind="ExternalInput").ap()
    lp = nc.dram_tensor("lp", [2, 128, 128], BF16, kind="ExternalInput").ap()
    hsel = nc.dram_tensor("hsel", [128, 2], BF16, kind="ExternalInput").ap()
    wv = nc.dram_tensor("wv", [8, 128, 256], BF16, kind="ExternalInput").ap()
    wvb = nc.dram_tensor("wvb", [1, 256], BF16, kind="ExternalInput").ap()
    onescol = nc.dram_tensor("onescol", [1, 128], BF16, kind="ExternalInput").ap()
    wo = nc.dram_tensor("wo", [2, 128, E], BF16, kind="ExternalInput").ap()
    fours = nc.dram_tensor("fours", [2, NT, 2, 128], F8, kind="ExternalInput").ap()
    zeros8 = nc.dram_tensor("zeros8", [64, S], F8, kind="ExternalInput").ap()
    partial = nc.dram_tensor("partial", [S, E], BF16, kind="ExternalOutput").ap()

    flags = _dve_flags()

    with tile.TileContext(nc) as tc:
        with (
            tc.tile_pool(name="consts", bufs=1) as consts,
            tc.tile_pool(name="stream", bufs=1) as streams,
        ):
            lp_t = consts.tile([128, 2, 128], BF16)
            hsel_t = consts.tile([128, 2], BF16)
            onescol_t = consts.tile([1, 128], BF16)
            wvb_t = consts.tile([1, 256], BF16)
            wv_t = consts.tile([128, 8, 256], BF16)
            wo_t = consts.tile([128, 2, E], BF16)
            xt_t = consts.tile([128, 8, S], BF16)
            # critical-path first: chunks 0-1 (z matmul) + lp/hsel (z + M)
            for k in range(2):
                nc.sync.dma_start(out=xt_t[:, k, :], in_=xt[k, :, :])
            for j in range(2):
                nc.sync.dma_start(out=lp_t[:, j, :], in_=lp[j, :, :])
            nc.sync.dma_start(out=hsel_t[:], in_=hsel[:])

            # fp8 scores operands: stream sA[h] and weights wT[h]
            sA = [streams.tile([128, 4, 2, 512], F8, name=f"sA{i}") for i in range(HPC)]
            wT = [streams.tile([128, NT, 128], F8, name=f"wT{i}") for i in range(HPC)]
            v_all = consts.tile([128, NT, HPC, 65], BF16)
            nc.gpsimd.memset(v_all[:, :, :, 64:65], 1.0)

            outnP = [consts.tile([128, NT, 2, 64], BF16, name=f"outnP{i}") for i in range(2)]
            onT = [consts.tile([128, NT, 128], BF16, name=f"onT{i}") for i in range(2)]

            # ---- Phase 0: z, M, fp8 quantize, operand assembly ----
            with (
                tc.tile_pool(name="zp", bufs=1, space="PSUM") as zp,
                tc.tile_pool(name="mp", bufs=1, space="PSUM") as mp,
                tc.tile_pool(name="q8", bufs=2) as q8,
                tc.tile_pool(name="sq", bufs=2) as sqp,
                tc.tile_pool(name="m8", bufs=2) as m8p,
            ):
                for j in range(2):
                    zp_t = zp.tile([128, S], F32, tag="zp")
                    for qc in range(4):
                        ql = slice(512 * qc, 512 * (qc + 1))
                        nc.tensor.matmul(zp_t[:, ql], lp_t[:, j, :],
                                         xt_t[:, j, ql], start=True, stop=True)
                    sq_t = sqp.tile([128, S], BF16, tag="sq")
                    mp_t = mp.tile([2, S], F32, tag="mp")
                    nt_t = sqp.tile([2, S], F32, tag="nt")
                    mh_t = m8p.tile([2, S], F8, tag="mh")
                    ml_t = m8p.tile([2, S], F8, tag="ml")
                    for qc in range(4):
                        ql = slice(512 * qc, 512 * (qc + 1))
                        nc.scalar.activation(sq_t[:, ql], zp_t[:, ql],
                                             ACT.Square)
                        nc.tensor.matmul(mp_t[:, ql], hsel_t[:], sq_t[:, ql],
                                         start=True, stop=True)
                        nc.vector.tensor_scalar(nt_t[:, ql], mp_t[:, ql],
                                                -0.25, -MARGIN / 4.0,
                                                ALU.mult, ALU.add)
                        nc.scalar.activation(mh_t[:, ql], mp_t[:, ql],
                                             ACT.Copy, bias=-MARGIN / 4.0,
                                             scale=-0.25)
                        nc.gpsimd.tensor_tensor(ml_t[:, ql], nt_t[:, ql],
                                                mh_t[:, ql], ALU.subtract)
                    zq_t = q8.tile([128, S], F8, tag="zq")
                    nc.scalar.activation(zq_t[:], zp_t[:], ACT.Copy)
                    zl_t = q8.tile([128, S], F8, tag="zl")
                    nc.vector.tensor_tensor(zl_t[:], zp_t[:], zq_t[:],
                                            ALU.subtract)
                    def fanout(h, i, zq_t=None, zl_t=None, mh_t=None, ml_t=None):
                        r = 64 * i

                        def q4(ap_in):  # [P, 2048] -> [P, 4, 512]
                            return ap_in.rearrange("p (a b) -> p a b", a=4)

                        def t16(ap_in):  # [P, 2048] -> [P, 16, 128]
                            return ap_in.rearrange("p (a b) -> p a b", a=NT)

                        eng = nc.sync
                        eng.dma_start(out=sA[h][0:64, :, 0, :], in_=q4(zq_t[r:r + 64, :]))
                        eng.dma_start(out=sA[h][64:126, :, 0, :], in_=q4(zq_t[r:r + 62, :]))
                        eng.dma_start(out=sA[h][126:127, :, 0, :], in_=q4(mh_t[i:i + 1, :]))
                        eng.dma_start(out=sA[h][127:128, :, 0, :], in_=q4(ml_t[i:i + 1, :]))
                        eng.dma_start(out=sA[h][0:64, :, 1, :], in_=q4(zl_t[r:r + 64, :]))
                        eng.dma_start(out=sA[h][64:128, :, 1, :], in_=q4(zeros8[:]))

                        eng.dma_start(out=wT[h][0:64, :, :], in_=t16(zq_t[r:r + 64, :]))
                        eng.dma_start(out=wT[h][64:126, :, :], in_=t16(zl_t[r:r + 62, :]))
                        eng.dma_start(out=wT[h][126:128, :, :], in_=fours[:, :, 0, :])

                    ctx = dict(zq_t=zq_t, zl_t=zl_t, mh_t=mh_t, ml_t=ml_t)
                    fanout(2 * j, 0, **ctx)
                    if j == 0:
                        # bulk loads ride behind h0's critical operands
                        for k in range(2, 8):
                            nc.sync.dma_start(out=xt_t[:, k, :], in_=xt[k, :, :])
                        nc.sync.dma_start(out=wv_t[:], in_=wv.rearrange("k p n -> p k n"))
                        nc.sync.dma_start(out=wvb_t[:], in_=wvb[:])
                        nc.sync.dma_start(out=onescol_t[:], in_=onescol[:])
                    fanout(2 * j + 1, 1, **ctx)
                    if j == 1:
                        for c in range(2):
                            nc.sync.dma_start(out=wo_t[:, c, :], in_=wo[c, :, :])

            # ---- Main: scores/exp/attnV per head (V-proj rides in h0) ----
            import contextlib

            with (
                tc.tile_pool(name="sp", bufs=3, space="PSUM") as sp,
                tc.tile_pool(name="ap", bufs=1, space="PSUM") as ap,
                tc.tile_pool(name="ep", bufs=34) as ep,
                tc.tile_pool(name="up", bufs=34) as up,
                tc.tile_pool(name="nrm", bufs=2) as nrm,
            ):
                def v_proj(t, vp):
                    vp_t = vp.tile([128, 256], F32, tag="vp")
                    tl = slice(128 * t, 128 * (t + 1))
                    for k in range(8):
                        nc.tensor.matmul(vp_t[:], xt_t[:, k, tl],
                                         wv_t[:, k, :], start=(k == 0),
                                         stop=False)
                    nc.tensor.matmul(vp_t[:], onescol_t[:], wvb_t[:],
                                     start=False, stop=True)
                    nc.vector.tensor_copy(
                        v_all[:, t, :, 0:64],
                        vp_t[:].rearrange("p (b c) -> p b c", b=4))

                def run_head(h, vp, spx):
                    ap_t = ap.tile([128, NT, 65], F32, tag="ap")

                    def attn_v(e_aps, t):
                        for qb in range(NT):
                            qsl = slice(128 * (qb % 4), 128 * (qb % 4 + 1))
                            nc.tensor.matmul(
                                ap_t[:, qb, :], e_aps[qb // 4][:, qsl],
                                v_all[:, t, h, :],
                                start=False, stop=(t == NT - 1),
                                skip_group_check=True)

                    pend = None
                    for t in range(NT):
                        e_aps = []
                        for qc in range(4):
                            pool = sp if (spx is None or qc < 2) else spx
                            spt = pool.tile([128, 512], F32, tag="sp")
                            wT_b = wT[h][:, t, :].unsqueeze(1).broadcast_to(
                                [128, 2, 128])
                            nc.tensor.matmul(
                                spt[:], wT_b, sA[h][:, qc, :, :],
                                start=True, stop=True,
                                perf_mode=mybir.MatmulPerfMode.DoubleRow)
                            if qc % 2 == 1:
                                u_t = up.tile([128, 512], U16, tag="u")
                                nc.vector.tensor_scalar(u_t[:], spt[:],
                                                        SCH_SCALE, SCH_BIAS,
                                                        ALU.mult, ALU.add)
                                e_aps.append(u_t[:].bitcast(BF16))
                            else:
                                e_t = ep.tile([128, 512], BF16, tag="e")
                                nc.scalar.activation(e_t[:], spt[:], ACT.Exp)
                                e_aps.append(e_t[:])
                        if t == 0:
                            nc.scalar.activation(ap_t[:], ap_t[:],
                                                 ACT.Copy, scale=0.0)
                        if h == 0:
                            v_proj(t, vp)
                        if pend is not None:
                            attn_v(*pend)
                        pend = (e_aps, t)
                    attn_v(*pend)

                    rc_t = nrm.tile([128, NT, 1], F32, tag="rc")
                    nc.vector.reciprocal(rc_t[:], ap_t[:, :, 64:65])
                    hp, hi = h // 2, h % 2
                    rc_b = rc_t[:].broadcast_to([128, NT, 64])
                    nc.vector.tensor_tensor(
                        outnP[hp][:, :, hi, :], ap_t[:, :, 0:64], rc_b,
                        ALU.mult)
                    if hi == 1:
                        nc.sync.dma_start_transpose(
                            onT[hp][:],
                            outnP[hp][:].rearrange("p a b c -> p (a b c)"))

                with tc.tile_pool(name="vp", bufs=2, space="PSUM") as vp:
                    run_head(0, vp, None)
                with tc.tile_pool(name="spx", bufs=2, space="PSUM") as spx:
                    for h in range(1, HPC):
                        run_head(h, None, spx)

            # ---- Tail: Wo projection + store ----
            with (
                tc.tile_pool(name="wp", bufs=4, space="PSUM") as wp,
                tc.tile_pool(name="ws", bufs=6) as ws,
            ):
                for qb in range(NT):
                    wp_t = wp.tile([128, E], F32, tag="wp")
                    for c2 in range(2):
                        cl = slice(512 * c2, 512 * (c2 + 1))
                        for hp in range(2):
                            nc.tensor.matmul(wp_t[:, cl], onT[hp][:, qb, :],
                                             wo_t[:, hp, cl], start=(hp == 0),
                                             stop=(hp == 1))
                    ws_t = ws.tile([128, E], BF16, tag="ws")
                    nc.scalar.activation(ws_t[:, 0:512], wp_t[:, 0:512],
                                         ACT.Copy)
                    nc.vector.tensor_copy(ws_t[:, 512:1024], wp_t[:, 512:1024])
                    nc.sync.dma_start(out=partial[128 * qb:128 * (qb + 1), :],
                                      in_=ws_t[:])

    _split_multi_waits(nc)
    return nc


_prog_cache = {}


def _get_program():
    if "nc" not in _prog_cache:
        _prog_cache["nc"] = build_program()
    return _prog_cache["nc"]


def make_in_maps(x, params, Wv, bv, Wo, bo):
    x = np.asarray(x, np.float32)
    params = np.asarray(params, np.float32)
    Wv = np.asarray(Wv, np.float32)
    bv = np.asarray(bv, np.float32)
    Wo = np.asarray(Wo, np.float32)

    rows, cols = np.tril_indices(D)
    L = np.zeros((H, D, D), np.float32)
    L[:, rows, cols] = params
    Ls = (L / np.float32(np.sqrt(8.0)))

    hsel = np.zeros((128, 2), np.float32)
    hsel[0:64, 0] = 1.0
    hsel[64:128, 1] = 1.0
    onescol = np.ones((1, 128), np.float32)
    fours = np.full((2, NT, 2, 128), 4.0, np.float32).astype(NPF8)
    zeros8 = np.zeros((64, S), np.float32).astype(NPF8)

    xT = [np.ascontiguousarray(x[b].T) for b in range(B)]

    in_maps = []
    for m in range(NCORES):
        b = m // 4
        hbase = HPC * (m % 4)
        heads = list(range(hbase, hbase + HPC))
        own = list(range(hbase * D, (hbase + HPC) * D))
        rest = [e for e in range(E) if not (hbase * D <= e < (hbase + HPC) * D)]
        perm = own + rest
        xt_m = xT[b][perm, :].reshape(8, 128, S).astype(NPBF)

        lp_m = np.zeros((2, 128, 128), np.float32)
        for j in range(2):
            lp_m[j, 0:64, 0:64] = Ls[heads[2 * j]]
            lp_m[j, 64:128, 64:128] = Ls[heads[2 * j + 1]]
        lp_m = lp_m.astype(NPBF)

        # wv: [8, 128, 256]; col block hl = Wv_head^T rows (permuted)
        wv_m = np.zeros((E, 256), np.float32)
        for i, h in enumerate(heads):
            wv_m[:, 64 * i:64 * i + 64] = Wv[h * D:(h + 1) * D, perm].T
        wv_m = wv_m.reshape(8, 128, 256).astype(NPBF)
        wvb_m = np.concatenate(
            [bv[h * D:(h + 1) * D] for h in heads]).reshape(1, 256).astype(NPBF)

        wo_m = np.stack([
            np.ascontiguousarray(Wo[:, (hbase + 2 * c) * D:(hbase + 2 * c + 2) * D].T)
            for c in range(2)]).astype(NPBF)

        in_maps.append({
            "xt": xt_m, "lp": lp_m, "hsel": hsel.astype(NPBF),
            "wv": wv_m, "wvb": wvb_m, "onescol": onescol.astype(NPBF),
            "wo": wo_m, "fours": fours, "zeros8": zeros8,
        })
    return in_maps


def run(x, params, Wv, bv, Wo, bo, trace=False):
    nc = _get_program()
    in_maps = make_in_maps(x, params, Wv, bv, Wo, bo)
    r = run_bass_kernel_spmd(nc, in_maps, list(range(NCORES)), trace=trace)
    bo = np.asarray(bo, np.float32)
    out = np.zeros((B, S, E), np.float32)
    for b in range(B):
        acc = np.zeros((S, E), np.float64)
        for m in range(4 * b, 4 * b + 4):
            acc += r.results[m]["partial"].astype(np.float64)
        out[b] = (acc + bo).astype(np.float32)
    return out, r


def kernel(x, params, Wv, bv, Wo, bo):
    out, _ = run(x, params, Wv, bv, Wo, bo, trace=False)
    return out


# revision 51
# speedup vs baseline: 1.0033x; 1.0033x over previous
"""Trainium2 Bass kernel for nn_MultiHeadSelfAttention_90537910600353.

Reference (B=2, S=2048, E=1024, H=16, d=64):
    L_h   = tril(params[h]);  scores = (x_h Ls)(x_h Ls)^T,  Ls = L/sqrt(8)
    attn  = softmax(scores);  V = x Wv^T + bv
    out   = (attn @ V_h) @ Wo^T + bo

Sharding: batch*head over 8 cores; core m: b = m//4, heads [4(m%4), 4(m%4)+4).
Each core emits partial[S, E] (bf16); host sums 4 partials/batch + bo.

Per-core algorithm (all engine work balanced against the PE roofline):
  z      = x_h Ls                      (bf16 matmul, PSUM f32)
  zh/zl  = fp8e4m3 pair split of z     (Act copy + DVE subtract)
  M      = |z|^2 + 24 via Square + ones-matmul; fp8 pair -(M)/4 rows
  scores = fp8 DoubleRow matmul (0.5 cyc/row): slot0 = zh.zh + zl'.zh - M,
           slot1 = zh.zl  -> exact-ish 3-term pair product
  E      = exp(scores) -> bf16; split between Act (true exp) and DVE
           (Schraudolph uint16 bit-trick exp, validated on silicon)
  attnV  = weights-stationary matmuls: lhsT = E tile [t,q], rhs = V'[t, 65]
           (aug ones col -> denominator), accumulate over t in PSUM [q, 16, 65]
  outn   = num * recip(den) per partition (q) -> bf16 [q, qb, h%2, d]
  onT    = DMA-xbar blocked transpose -> [hd, qb, q]
  partial= onT^T @ Wo^T (bf16) -> PSUM -> bf16 -> DRAM

Self-contained: numpy + ml_dtypes + concourse at /opt/trn_rl_repo.
"""

import sys

if "/opt/trn_rl_repo" not in sys.path:
    sys.path.insert(0, "/opt/trn_rl_repo")

import numpy as np
import ml_dtypes

import concourse.bass as bass
import concourse.mybir as mybir
import concourse.tile as tile
from concourse.bass_utils import run_bass_kernel_spmd

F32 = mybir.dt.float32
BF16 = mybir.dt.bfloat16
F8 = mybir.dt.float8e4
U16 = mybir.dt.uint16
NPF8 = ml_dtypes.float8_e4m3
NPBF = ml_dtypes.bfloat16

B, S, E, H = 2, 2048, 1024, 16
D = 64
NCORES = 8
HPC = 4
NT = S // 128       # 16 t-blocks
MARGIN = 24.0

SCH_SCALE = float(np.log2(np.e) * 128.0)
SCH_BIAS = float(127 * 128 - 10.0)
Y_DVE = 56          # of 128 exp spans go to DVE (Schraudolph)

ACT = mybir.ActivationFunctionType
ALU = mybir.AluOpType


def _split_multi_waits(nc):
    """This walrus build rejects instructions carrying more than one sync
    wait. Hoist extra waits onto same-engine NOPs inserted just before."""
    for f in nc.m.functions:
        for b in f.blocks:
            il = b.instructions
            i = 0
            while i < len(il):
                inst = il[i]
                si = getattr(inst, "sync_info", None)
                if si is not None and si.on_wait and len(si.on_wait) > 1:
                    waits = list(si.on_wait)
                    for w in waits[:-1]:
                        nop = mybir.InstNoOp(
                            name=nc.get_next_instruction_name(),
                            engine=inst.engine,
                            sync_info=mybir.SyncInfo(on_wait=[w], on_update=[]),
                        )
                        il.insert(i, nop)
                        i += 1
                    inst.sync_info = mybir.SyncInfo(
                        on_wait=[waits[-1]], on_update=si.on_update
                    )
                i += 1


def _dve_flags():
    """Bresenham spread of Y_DVE Schraudolph spans over the 128 (h,t,s)."""
    return [((i + 1) * Y_DVE) // 128 - (i * Y_DVE) // 128 == 1 for i in range(128)]


def build_program():
    nc = bass.Bass("TRN2", target_bir_lowering=False, debug=False,
                   num_devices=NCORES)

    xt = nc.dram_tensor("xt", [8, 128, S], BF16, kind="ExternalInput").ap()
    lp = nc.dram_tensor("lp", [2, 128, 128], BF16, kind="ExternalInput").ap()
    hsel = nc.dram_tensor("hsel", [128, 2], BF16, kind="ExternalInput").ap()
    wv = nc.dram_tensor("wv", [8, 128, 256], BF16, kind="ExternalInput").ap()
    wvb = nc.dram_tensor("wvb", [1, 256], BF16, kind="ExternalInput").ap()
    onescol = nc.dram_tensor("onescol", [1, 128], BF16, kind="ExternalInput").ap()
    wo = nc.dram_tensor("wo", [2, 128, E], BF16, kind="ExternalInput").ap()
    fours = nc.dram_tensor("fours", [2, NT, 2, 128], F8, kind="ExternalInput").ap()
    zeros8 = nc.dram_tensor("zeros8", [64, S], F8, kind="ExternalInput").ap()
    partial = nc.dram_tensor("partial", [S, E], BF16, kind="ExternalOutput").ap()

    flags = _dve_flags()

    with tile.TileContext(nc) as tc:
        with (
            tc.tile_pool(name="consts", bufs=1) as consts,
            tc.tile_pool(name="stream", bufs=1) as streams,
        ):
            lp_t = consts.tile([128, 2, 128], BF16)
            hsel_t = consts.tile([128, 2], BF16)
            onescol_t = consts.tile([1, 128], BF16)
            wvb_t = consts.tile([1, 256], BF16)
            wv_t = consts.tile([128, 8, 256], BF16)
            wo_t = consts.tile([128, 2, E], BF16)
            xt_t = consts.tile([128, 8, S], BF16)
            # critical-path first: chunks 0-1 (z matmul) + lp/hsel (z + M)
            for k in range(2):
                nc.sync.dma_start(out=xt_t[:, k, :], in_=xt[k, :, :])
            for j in range(2):
                nc.sync.dma_start(out=lp_t[:, j, :], in_=lp[j, :, :])
            nc.sync.dma_start(out=hsel_t[:], in_=hsel[:])

            # fp8 scores operands: stream sA[h] and weights wT[h]
            sA = [streams.tile([128, 4, 2, 512], F8, name=f"sA{i}") for i in range(HPC)]
            wT = [streams.tile([128, NT, 128], F8, name=f"wT{i}") for i in range(HPC)]
            v_all = consts.tile([128, NT, HPC, 65], BF16)
            nc.gpsimd.memset(v_all[:, :, :, 64:65], 1.0)

            outnP = [consts.tile([128, NT, 2, 64], BF16, name=f"outnP{i}") for i in range(2)]
            onT = [consts.tile([128, NT, 128], BF16, name=f"onT{i}") for i in range(2)]

            # ---- Phase 0: z, M, fp8 quantize, operand assembly ----
            with (
                tc.tile_pool(name="zp", bufs=1, space="PSUM") as zp,
                tc.tile_pool(name="mp", bufs=1, space="PSUM") as mp,
                tc.tile_pool(name="q8", bufs=2) as q8,
                tc.tile_pool(name="sq", bufs=2) as sqp,
                tc.tile_pool(name="m8", bufs=2) as m8p,
            ):
                for j in range(2):
                    zp_t = zp.tile([128, S], F32, tag="zp")
                    for qc in range(4):
                        ql = slice(512 * qc, 512 * (qc + 1))
                        nc.tensor.matmul(zp_t[:, ql], lp_t[:, j, :],
                                         xt_t[:, j, ql], start=True, stop=True)
                    sq_t = sqp.tile([128, S], BF16, tag="sq")
                    mp_t = mp.tile([2, S], F32, tag="mp")
                    nt_t = sqp.tile([2, S], F32, tag="nt")
                    mh_t = m8p.tile([2, S], F8, tag="mh")
                    ml_t = m8p.tile([2, S], F8, tag="ml")
                    for qc in range(4):
                        ql = slice(512 * qc, 512 * (qc + 1))
                        nc.scalar.activation(sq_t[:, ql], zp_t[:, ql],
                                             ACT.Square)
                        nc.tensor.matmul(mp_t[:, ql], hsel_t[:], sq_t[:, ql],
                                         start=True, stop=True)
                        nc.vector.tensor_scalar(nt_t[:, ql], mp_t[:, ql],
                                                -0.25, -MARGIN / 4.0,
                                                ALU.mult, ALU.add)
                        nc.scalar.activation(mh_t[:, ql], mp_t[:, ql],
                                             ACT.Copy, bias=-MARGIN / 4.0,
                                             scale=-0.25)
                        nc.gpsimd.tensor_tensor(ml_t[:, ql], nt_t[:, ql],
                                                mh_t[:, ql], ALU.subtract)
                    zq_t = q8.tile([128, S], F8, tag="zq")
                    nc.scalar.activation(zq_t[:], zp_t[:], ACT.Copy)
                    zl_t = q8.tile([128, S], F8, tag="zl")
                    nc.vector.tensor_tensor(zl_t[:], zp_t[:], zq_t[:],
                                            ALU.subtract)
                    def fanout(h, i, zq_t=None, zl_t=None, mh_t=None, ml_t=None):
                        r = 64 * i

                        def q4(ap_in):  # [P, 2048] -> [P, 4, 512]
                            return ap_in.rearrange("p (a b) -> p a b", a=4)

                        def t16(ap_in):  # [P, 2048] -> [P, 16, 128]
                            return ap_in.rearrange("p (a b) -> p a b", a=NT)

                        eng = nc.sync
                        eng.dma_start(out=sA[h][0:64, :, 0, :], in_=q4(zq_t[r:r + 64, :]))
                        eng.dma_start(out=sA[h][64:126, :, 0, :], in_=q4(zq_t[r:r + 62, :]))
                        eng.dma_start(out=sA[h][126:127, :, 0, :], in_=q4(mh_t[i:i + 1, :]))
                        eng.dma_start(out=sA[h][127:128, :, 0, :], in_=q4(ml_t[i:i + 1, :]))
                        eng.dma_start(out=sA[h][0:64, :, 1, :], in_=q4(zl_t[r:r + 64, :]))
                        eng.dma_start(out=sA[h][64:128, :, 1, :], in_=q4(zeros8[:]))

                        eng.dma_start(out=wT[h][0:64, :, :], in_=t16(zq_t[r:r + 64, :]))
                        eng.dma_start(out=wT[h][64:126, :, :], in_=t16(zl_t[r:r + 62, :]))
                        eng.dma_start(out=wT[h][126:128, :, :], in_=fours[:, :, 0, :])

                    ctx = dict(zq_t=zq_t, zl_t=zl_t, mh_t=mh_t, ml_t=ml_t)
                    fanout(2 * j, 0, **ctx)
                    if j == 0:
                        # bulk loads ride behind h0's critical operands
                        for k in range(2, 8):
                            nc.sync.dma_start(out=xt_t[:, k, :], in_=xt[k, :, :])
                        nc.sync.dma_start(out=wv_t[:], in_=wv.rearrange("k p n -> p k n"))
                        nc.sync.dma_start(out=wvb_t[:], in_=wvb[:])
                        nc.sync.dma_start(out=onescol_t[:], in_=onescol[:])
                    fanout(2 * j + 1, 1, **ctx)
                    if j == 1:
                        for c in range(2):
                            nc.sync.dma_start(out=wo_t[:, c, :], in_=wo[c, :, :])

            # ---- Main: scores/exp/attnV per head (V-proj rides in h0) ----
            import contextlib

            with (
                tc.tile_pool(name="sp", bufs=3, space="PSUM") as sp,
                tc.tile_pool(name="ap", bufs=1, space="PSUM") as ap,
                tc.tile_pool(name="ep", bufs=34) as ep,
                tc.tile_pool(name="up", bufs=34) as up,
                tc.tile_pool(name="nrm", bufs=2) as nrm,
            ):
                def v_proj(t, vp):
                    vp_t = vp.tile([128, 256], F32, tag="vp")
                    tl = slice(128 * t, 128 * (t + 1))
                    for k in range(8):
                        nc.tensor.matmul(vp_t[:], xt_t[:, k, tl],
                                         wv_t[:, k, :], start=(k == 0),
                                         stop=False)
                    nc.tensor.matmul(vp_t[:], onescol_t[:], wvb_t[:],
                                     start=False, stop=True)
                    eng = nc.scalar if t % 2 == 0 else nc.vector
                    if t % 2 == 0:
                        nc.scalar.activation(
                            v_all[:, t, :, 0:64],
                            vp_t[:].rearrange("p (b c) -> p b c", b=4),
                            ACT.Copy)
                    else:
                        nc.vector.tensor_copy(
                            v_all[:, t, :, 0:64],
                            vp_t[:].rearrange("p (b c) -> p b c", b=4))

                def run_head(h, vp, spx):
                    ap_t = ap.tile([128, NT, 65], F32, tag="ap")

                    def attn_v(e_aps, t):
                        for qb in range(NT):
                            qsl = slice(128 * (qb % 4), 128 * (qb % 4 + 1))
                            nc.tensor.matmul(
                                ap_t[:, qb, :], e_aps[qb // 4][:, qsl],
                                v_all[:, t, h, :],
                                start=False, stop=(t == NT - 1),
                                skip_group_check=True)

                    pend = None
                    for t in range(NT):
                        e_aps = []
                        for qc in range(4):
                            pool = sp if (spx is None or qc < 2) else spx
                            spt = pool.tile([128, 512], F32, tag="sp")
                            wT_b = wT[h][:, t, :].unsqueeze(1).broadcast_to(
                                [128, 2, 128])
                            nc.tensor.matmul(
                                spt[:], wT_b, sA[h][:, qc, :, :],
                                start=True, stop=True,
                                perf_mode=mybir.MatmulPerfMode.DoubleRow)
                            if qc % 2 == 1:
                                u_t = up.tile([128, 512], U16, tag="u")
                                nc.vector.tensor_scalar(u_t[:], spt[:],
                                                        SCH_SCALE, SCH_BIAS,
                                                        ALU.mult, ALU.add)
                                e_aps.append(u_t[:].bitcast(BF16))
                            else:
                                e_t = ep.tile([128, 512], BF16, tag="e")
                                nc.scalar.activation(e_t[:], spt[:], ACT.Exp)
                                e_aps.append(e_t[:])
                        if t == 0:
                            nc.scalar.activation(ap_t[:], ap_t[:],
                                                 ACT.Copy, scale=0.0)
                        if h == 0:
                            v_proj(t, vp)
                        if pend is not None:
                            attn_v(*pend)
                        pend = (e_aps, t)
                    attn_v(*pend)

                    rc_t = nrm.tile([128, NT, 1], F32, tag="rc")
                    nc.vector.reciprocal(rc_t[:], ap_t[:, :, 64:65])
                    hp, hi = h // 2, h % 2
                    rc_b = rc_t[:].broadcast_to([128, NT, 64])
                    nc.vector.tensor_tensor(
                        outnP[hp][:, :, hi, :], ap_t[:, :, 0:64], rc_b,
                        ALU.mult)
                    if hi == 1:
                        nc.sync.dma_start_transpose(
                            onT[hp][:],
                            outnP[hp][:].rearrange("p a b c -> p (a b c)"))

                with tc.tile_pool(name="vp", bufs=2, space="PSUM") as vp:
                    run_head(0, vp, None)
                with tc.tile_pool(name="spx", bufs=2, space="PSUM") as spx:
                    for h in range(1, HPC):
                        run_head(h, None, spx)

            # ---- Tail: Wo projection + store ----
            with (
                tc.tile_pool(name="wp", bufs=4, space="PSUM") as wp,
                tc.tile_pool(name="ws", bufs=6) as ws,
            ):
                for qb in range(NT):
                    wp_t = wp.tile([128, E], F32, tag="wp")
                    for c2 in range(2):
                        cl = slice(512 * c2, 512 * (c2 + 1))
                        for hp in range(2):
                            nc.tensor.matmul(wp_t[:, cl], onT[hp][:, qb, :],
                                             wo_t[:, hp, cl], start=(hp == 0),
                                             stop=(hp == 1))
                    ws_t = ws.tile([128, E], BF16, tag="ws")
                    nc.scalar.activation(ws_t[:, 0:512], wp_t[:, 0:512],
                                         ACT.Copy)
                    nc.vector.tensor_copy(ws_t[:, 512:1024], wp_t[:, 512:1024])
                    nc.sync.dma_start(out=partial[128 * qb:128 * (qb + 1), :],
                                      in_=ws_t[:])

    _split_multi_waits(nc)
    return nc


_prog_cache = {}


def _get_program():
    if "nc" not in _prog_cache:
        _prog_cache["nc"] = build_program()
    return _prog_cache["nc"]


def make_in_maps(x, params, Wv, bv, Wo, bo):
    x = np.asarray(x, np.float32)
    params = np.asarray(params, np.float32)
    Wv = np.asarray(Wv, np.float32)
    bv = np.asarray(bv, np.float32)
    Wo = np.asarray(Wo, np.float32)

    rows, cols = np.tril_indices(D)
    L = np.zeros((H, D, D), np.float32)
    L[:, rows, cols] = params
    Ls = (L / np.float32(np.sqrt(8.0)))

    hsel = np.zeros((128, 2), np.float32)
    hsel[0:64, 0] = 1.0
    hsel[64:128, 1] = 1.0
    onescol = np.ones((1, 128), np.float32)
    fours = np.full((2, NT, 2, 128), 4.0, np.float32).astype(NPF8)
    zeros8 = np.zeros((64, S), np.float32).astype(NPF8)

    xT = [np.ascontiguousarray(x[b].T) for b in range(B)]

    in_maps = []
    for m in range(NCORES):
        b = m // 4
        hbase = HPC * (m % 4)
        heads = list(range(hbase, hbase + HPC))
        own = list(range(hbase * D, (hbase + HPC) * D))
        rest = [e for e in range(E) if not (hbase * D <= e < (hbase + HPC) * D)]
        perm = own + rest
        xt_m = xT[b][perm, :].reshape(8, 128, S).astype(NPBF)

        lp_m = np.zeros((2, 128, 128), np.float32)
        for j in range(2):
            lp_m[j, 0:64, 0:64] = Ls[heads[2 * j]]
            lp_m[j, 64:128, 64:128] = Ls[heads[2 * j + 1]]
        lp_m = lp_m.astype(NPBF)

        # wv: [8, 128, 256]; col block hl = Wv_head^T rows (permuted)
        wv_m = np.zeros((E, 256), np.float32)
        for i, h in enumerate(heads):
            wv_m[:, 64 * i:64 * i + 64] = Wv[h * D:(h + 1) * D, perm].T
        wv_m = wv_m.reshape(8, 128, 256).astype(NPBF)
        wvb_m = np.concatenate(
            [bv[h * D:(h + 1) * D] for h in heads]).reshape(1, 256).astype(NPBF)

        wo_m = np.stack([
            np.ascontiguousarray(Wo[:, (hbase + 2 * c) * D:(hbase + 2 * c + 2) * D].T)
            for c in range(2)]).astype(NPBF)

        in_maps.append({
            "xt": xt_m, "lp": lp_m, "hsel": hsel.astype(NPBF),
            "wv": wv_m, "wvb": wvb_m, "onescol": onescol.astype(NPBF),
            "wo": wo_m, "fours": fours, "zeros8": zeros8,
        })
    return in_maps


def run(x, params, Wv, bv, Wo, bo, trace=False):
    nc = _get_program()
    in_maps = make_in_maps(x, params, Wv, bv, Wo, bo)
    r = run_bass_kernel_spmd(nc, in_maps, list(range(NCORES)), trace=trace)
    bo = np.asarray(bo, np.float32)
    out = np.zeros((B, S, E), np.float32)
    for b in range(B):
        acc = np.zeros((S, E), np.float64)
        for m in range(4 * b, 4 * b + 4):
            acc += r.results[m]["partial"].astype(np.float64)
        out[b] = (acc + bo).astype(np.float32)
    return out, r


def kernel(x, params, Wv, bv, Wo, bo):
    out, _ = run(x, params, Wv, bv, Wo, bo, trace=False)
    return out
